# revision 22
# baseline (speedup 1.0000x reference)
"""DigitCaps dynamic-routing kernel for 8 Trainium2 NeuronCores.

Strategy (v6): shard the routes dimension R=1024 across the 8 cores (128
routes per core).  u_hat is never materialized: each routing iteration
computes its weighted route-sum

    s[b,c,o] = sum_{r,i} x[b,r,i] * (c_ij[r,c] * W[r,c,o,i])

directly on the PE as 16 accumulating fp16 matmuls (W held i-major so the
moving operand is contiguous).  The per-core partial s is combined across
cores with a single fp16 AllReduce per routing round (iters 0,1) — the
CCE adds on the wire, so the readback is one 80KB download (split across
the sync and scalar HWDGE engines per batch half, letting the bc0 squash
start while bc1 is still landing) and no on-core fold tree is needed.
The final iteration uses an AllToAll; the 8 rank shards it delivers are
summed in ONE PE matmul against a p%16 one-hot mask, and each core
squashes only its own 32-batch output shard.  The agreement update

    agree[r,c] = sum_{b,o,i} W[r,c,o,i] * x[b,r,i] * v[b,c,o]

is computed locally via G[r,i,c,o] = sum_b x[b,r,i]*v[b,c,o] (PE matmuls;
per-PSUM-region start/stop pairs kept consecutive) followed by a multiply
that reads G straight out of PSUM (fp32) against the pre-transposed W and
a group-reduce on the vector engine.

Activation-table discipline: iters 0/1 compute sqrt(q) as exp(0.5*ln q)
so the scalar engine only ever needs the ln/exp table (shared with the
softmax exp).  x is supplied twice from the host (b-major and r-major) so
no on-device transposes are needed.
"""

import math
import sys

for _p in ("/opt/trn_rl_repo",):
    if _p not in sys.path:
        sys.path.insert(0, _p)

import numpy as np

import concourse.bass as bass
import concourse.bacc as bacc
import concourse.mybir as mybir
import concourse.tile as tile
from concourse.bass_utils import run_bass_kernel_spmd

F32 = mybir.dt.float32
F16 = mybir.dt.float16

B, R, C, O, I = 256, 1024, 10, 16, 8
NCORES = 8
RS = R // NCORES          # routes per core
CO = C * O                # 160
COI = C * O * I           # 1280
BS = B // NCORES          # output batch shard per core
NITER = 3
A0 = 0.1                  # softmax(0) over C=10 entries
# pacer counts: keep the PE HAM window busy through collective/DVE
# stretches.  Target (window - ~3.4us HAM grace); overshoot only delays
# the next real matmul by one dummy's tail (~110ns).
PACE_AR = 55              # mm-end .. G-bc0-start window (~10us modeled)
PACE_GAP = 18             # G-bc0 .. G-bc1 gap (~4.5us modeled)
PACE_MID = 30             # agree/softmax stretch (~4us modeled)
PACE_A2A = 0              # nothing runs on PE after the final mm block


def build_nc(reps=1, niter=NITER, fake_cc=False, chain=False, pace=True,
             pace_ar=PACE_AR, pace_mid=PACE_MID, pace_a2a=PACE_A2A, dbg=None):
    nc = bacc.Bacc(
        "TRN2", target_bir_lowering=False, debug=False, num_devices=NCORES
    )
    xt_d = nc.dram_tensor("xt", [RS, 2 * I * 128], F16, kind="ExternalInput")
    xs_d = nc.dram_tensor("xs", [128, 2 * I * RS], F16, kind="ExternalInput")
    # W in (i, c, o) layout: contiguous 160-wide moving operand per i
    ws_d = nc.dram_tensor("ws", [RS, COI], F16, kind="ExternalInput")
    # W in (c, i, o) layout: packed innermost match for the agree multiply
    wst_d = nc.dram_tensor("wst", [RS, COI], F16, kind="ExternalInput")
    # p%16 one-hot fold mask for the final-iteration rank-shard sum
    msk_d = nc.dram_tensor("msk", [128, 16], F16, kind="ExternalInput")
    # [16, 2*CO]: row r = batches (16k + r, 128 + 16k + r) for core k —
    # the A2A partition-shard layout; kernel() un-permutes on the host.
    out_d = nc.dram_tensor("vout", [16, 2 * CO], F32, kind="ExternalOutput")
    dbg_d = (nc.dram_tensor("dbg", [128, 2 * CO], F16, kind="ExternalOutput")
             if dbg else None)

    with tile.TileContext(nc) as tc:
        with (
            tc.tile_pool(name="main", bufs=1) as pool,
            tc.tile_pool(name="ps", bufs=1, space=bass.MemorySpace.PSUM) as ps,
            tc.tile_pool(name="pg", bufs=1, space=bass.MemorySpace.PSUM) as pg,
            tc.tile_pool(name="dram", bufs=1, space="DRAM") as dram,
        ):
            pools = (pool, ps, pg, dram)
            for rep in range(reps):
                _build_body(nc, tc, pools, xt_d, xs_d, ws_d, wst_d, msk_d, out_d, rep,
                            niter=niter, fake_cc=fake_cc,
                            chain=chain and rep > 0, pace=pace,
                            pace_ar=pace_ar, pace_mid=pace_mid,
                            pace_a2a=pace_a2a, dbg=dbg, dbg_d=dbg_d)
    nc.finalize()
    _unify_act_tables(nc)
    return nc


def _unify_act_tables(nc):
    """Point every activation-table load at the one table covering all the
    functions this kernel uses (ln, exp, copy), then drop redundant loads.

    The builtin insertion pass picks the first table containing each
    function (ln -> natural_log, exp -> exp_and_others), which forces a
    ~1.3us table reload at every ln<->exp transition on the scalar engine.
    All loads it inserts are dependency-free queue-order instructions, so
    rewriting ids and deleting duplicates is safe.
    """
    from concourse.hw_specs import get_activation_tables

    need = {
        mybir.ActivationFunctionType.Ln,
        mybir.ActivationFunctionType.Exp,
        mybir.ActivationFunctionType.Copy,
    }
    try:
        tabs = get_activation_tables(nc.m.arch)
        names = list(tabs)
        target = names.index("natural_log_exp_and_others")
        if not need <= tabs["natural_log_exp_and_others"]:
            return
    except Exception:
        # unknown act_info layout: leave the stock (slower) table loads
        return
    cur = None
    for bb in nc.main_func.blocks:
        keep = []
        for inst in bb.instructions:
            if isinstance(inst, mybir.InstLoadActFuncSet):
                inst.act_func_set_id = target
                if cur == target:
                    continue
                cur = target
            elif isinstance(inst, mybir.InstActivation):
                assert inst.func in need, f"unexpected ACT func {inst.func}"
            keep.append(inst)
        bb.instructions[:] = keep


def _build_body(nc, tc, pools, xt_d, xs_d, ws_d, wst_d, msk_d, out_d, rep, niter=NITER,
                fake_cc=False, chain=False, pace=True, pace_ar=PACE_AR,
                pace_mid=PACE_MID, pace_a2a=PACE_A2A, dbg=None, dbg_d=None):
    pool, ps, pg, dram = pools
    rg = [list(range(NCORES))]
    rp = f"r{rep}_"

    def _pace_pe(dummy_ps, xt, n):
        # Keep the PE p-state streak alive across collective/DVE windows:
        # self-paced throwaway matmuls into a scratch PSUM bank.  No
        # consumers; they only read xt, so they fill PE idle time without
        # delaying ready work by more than one dummy's tail (~110ns).
        for _ in range(n):
            nc.tensor.matmul(
                dummy_ps[:], xt[:, 0:128], xt[:, 0:256],
                start=True, stop=True, skip_group_check=True,
            )

    # ---------------- tiles ----------------
    xt = pool.tile([RS, 16 * 128], F16)       # [r, (bc i)*128 + b]
    xs = pool.tile([128, 2 * I * RS], F16)    # [b%128, bc*1024 + i*128 + r]
    ws = pool.tile([RS, COI], F16)            # [r, i*160 + c*16 + o]
    wst = pool.tile([RS, COI], F16)           # [r, c*128 + i*16 + o]
    wp = pool.tile([RS, COI], F16, name=f"{rp}wp", tag="wp")
    prm = pool.tile([1, 2], F32, name=f"{rp}prm", tag="prm")

    if chain:
        # Serialize this rep behind the previous one's final output: a tiny
        # DMA from out_d into xt creates a WAW overlap with the real xt
        # load, so timing reps measure end-to-end latency.
        poison = out_d[0:16, 0:20].bitcast(F16)
        nc.sync.dma_start(xt[0:16, 0 : poison.shape[1]], poison)

    # activation-table prime: first ACT instruction loads the ln/exp table
    nc.vector.memset(prm[:], 1.0)
    nc.scalar.activation(prm[0:1, 0:1], prm[0:1, 0:1],
                         mybir.ActivationFunctionType.Ln)

    # PE warm-up: ~8 throwaway matmuls on a zeroed tile lift the PE out of
    # its cold p-state while the first x/W DMAs land, so the t=0 matmul
    # block starts at speed (overshoot is bounded by one dummy's tail)
    wz = None
    if pace:
        wz = pool.tile([128, 256], F16, name=f"{rp}wz", tag="wz")
        nc.vector.memset(wz[:], 0.0)
        wz_ps = ps.tile([128, 256], F32, tag="dummy_ps", name=f"{rp}wz_ps")
        for _ in range(8):
            nc.tensor.matmul(
                wz_ps[:], wz[:, 0:128], wz[:, 0:256],
                start=True, stop=True, skip_group_check=True,
            )

    nc.sync.dma_start(ws[:], ws_d[:])
    for g in range(4):
        nc.sync.dma_start(
            xt[:, g * 512 : (g + 1) * 512], xt_d[:, g * 512 : (g + 1) * 512]
        )
    # prefetch the G/agree operands now: they stream during the t=0 mm
    # block (which only needs xt/ws) and are long done before AR#1 starts,
    # so the AllReduce window sees no competing SDMA/HBM traffic
    nc.sync.dma_start(xs[:], xs_d[:])
    nc.sync.dma_start(wst[:], wst_d[:])

    # fold mask for the final-iteration rank-shard sum on the PE:
    # mask[p, j] = 1 if p % 16 == j (supplied as a tiny host input)
    mask = pool.tile([128, 16], F16, name=f"{rp}mask", tag="mask")
    nc.sync.dma_start(mask[:], msk_d[:])

    w4 = ws[:].rearrange("p (i c o) -> p i c o", i=I, c=C, o=O)
    wp4 = wp[:].rearrange("p (i c o) -> p i c o", i=I, c=C, o=O)

    dummy_ps = (
        ps.tile([128, 256], F32, tag="dummy_ps", name=f"{rp}dummy_ps")
        if pace else None
    )

    # collective buffers (HBM).  The per-round reduce is ONE fp16
    # AllReduce of the [128, 2*CO] payload: the CCE sums the 8 rank
    # partials on the wire, so the readback is 80KB (vs 640KB for an
    # AllGather + on-core folds) and no fold tree is needed.
    ar_in = [dram.tile([128, 2 * CO], F16, name=f"{rp}ar_in{t}") for t in range(2)]
    ar_out = [
        dram.tile([128, 2 * CO], F16, name=f"{rp}ar_out{t}",
                  addr_space="Shared")
        for t in range(2)
    ]
    a2a_in = dram.tile([128, 2 * CO], F16, name=f"{rp}a2a_in")
    a2a_out = dram.tile([128, 2 * CO], F16, name=f"{rp}a2a_out")

    b_cum = pool.tile([RS, C], F32, name=f"{rp}bcum", tag="bcum")

    if niter == 0:
        nc.sync.dma_start(out_d[:], xt[0:16, 0 : 4 * CO].bitcast(F32))
        return

    for t in range(niter):
        last = t == niter - 1
        # ---- route-weighted sum matmuls ----
        rhs4 = w4 if t == 0 else wp4
        s_ps = [
            ps.tile([128, CO], F32, tag=f"s_ps{bc}", name=f"{rp}s_ps{bc}_{t}")
            for bc in range(2)
        ]
        for bc in range(2):
            for i in range(I):
                nc.tensor.matmul(
                    s_ps[bc][:],
                    xt[:, (bc * 8 + i) * 128 : (bc * 8 + i + 1) * 128],
                    rhs4[:, i],
                    start=(i == 0),
                    stop=(i == I - 1),
                )
        # stage PSUM -> fp16 SBUF on both vector and scalar so the two
        # halves cast in parallel
        cat = pool.tile([128, 2 * CO], F16, tag="cat", name=f"{rp}cat_{t}")
        nc.vector.tensor_copy(cat[:, 0:CO], s_ps[0][:])
        nc.scalar.copy(cat[:, CO : 2 * CO], s_ps[1][:])

        if not last:
            # ================= AllReduce (CCE adds on the wire) ========
            # upload bc0 on sync, bc1 on scalar: parallel descriptor
            # issue (one DMA_DIRECT2D costs ~0.6us of sequencer time)
            nc.sync.dma_start(ar_in[t][:, 0:CO], cat[:, 0:CO])
            nc.scalar.dma_start(ar_in[t][:, CO : 2 * CO], cat[:, CO : 2 * CO])
            if fake_cc:
                nc.sync.dma_start(ar_out[t][:], ar_in[t][:])
            else:
                nc.gpsimd.collective_compute(
                    "AllReduce",
                    mybir.AluOpType.add,
                    replica_groups=rg,
                    ins=[ar_in[t][:].opt()],
                    outs=[ar_out[t][:].opt()],
                )
            if pace and not fake_cc:
                _pace_pe(dummy_ps, xt, pace_ar)
            # download the summed s in bc halves on both DMA engines so
            # squash-bc0 starts while bc1 is still landing
            s_sb = pool.tile([128, 2 * CO], F16, tag="s_sb", name=f"{rp}ssb_{t}")
            nc.sync.dma_start(s_sb[:, 0:CO], ar_out[t][:, 0:CO])
            nc.scalar.dma_start(s_sb[:, CO : 2 * CO], ar_out[t][:, CO : 2 * CO])

            # per-bc: gather(sync+scalar halves) -> fold tree -> squash
            # -> v half -> G matmuls; bc0's chain hides under AG-bc1
            a2 = A0 * A0 if t == 0 else 1.0
            v_sb = pool.tile([128, 2 * CO], F16, tag="v_sb", name=f"{rp}v_{t}")
            if a2 != 1.0:
                lnb = pool.tile([128, 1], F32, name=f"{rp}lnb_{t}", tag="lnb")
                nc.vector.memset(lnb[:], float(math.log(a2)))
            g_ps = [
                pg.tile([128, 3 * CO], F32, tag=f"g_ps{gg}",
                        name=f"{rp}g_ps{gg}_{t}")
                for gg in range(3)
            ]
            for bc in range(2):
                sbc = s_sb[:, bc * CO : (bc + 1) * CO]
                # squash half: v = s * a2*sqrt(q)/(1 + a2*q), q = sum_o s^2
                tsq = pool.tile([128, CO], F32, tag=f"tsq_{bc}",
                                name=f"{rp}tsq_{bc}_{t}")
                q = pool.tile([128, C], F32, tag=f"sq_q{bc}",
                              name=f"{rp}q_{bc}_{t}")
                u2 = pool.tile([128, C], F32, tag=f"sq_u{bc}",
                               name=f"{rp}u_{bc}_{t}")
                den = pool.tile([128, C], F32, tag=f"sq_d{bc}",
                                name=f"{rp}d_{bc}_{t}")
                gf = pool.tile([128, C], F32, tag=f"sq_g{bc}",
                               name=f"{rp}g_{bc}_{t}")
                if dbg == f"s{t}":
                    nc.sync.dma_start(dbg_d[:, bc * CO : (bc + 1) * CO], sbc)
                nc.vector.tensor_mul(tsq[:], sbc, sbc)
                nc.vector.tensor_reduce(
                    q[:], tsq[:].rearrange("p (g o) -> p g o", o=O),
                    axis=mybir.AxisListType.X, op=mybir.AluOpType.add,
                )
                nc.scalar.activation(u2[:], q[:],
                                     mybir.ActivationFunctionType.Ln)
                nc.scalar.activation(
                    u2[:], u2[:], mybir.ActivationFunctionType.Exp,
                    bias=(0.0 if a2 == 1.0 else lnb[:]), scale=0.5,
                )
                nc.vector.tensor_scalar(
                    den[:], q[:], a2, 1.0, mybir.AluOpType.mult,
                    mybir.AluOpType.add,
                )
                rw = pool.tile([128, C], F32, tag=f"sq_r{bc}",
                               name=f"{rp}r_{bc}_{t}")
                nc.vector.reciprocal(rw[:], den[:])
                nc.vector.tensor_mul(gf[:], u2[:], rw[:])
                nc.vector.tensor_mul(
                    v_sb[:, bc * CO : (bc + 1) * CO].rearrange(
                        "p (g o) -> p g o", o=O
                    ),
                    sbc.rearrange("p (g o) -> p g o", o=O),
                    gf[:].unsqueeze(2).broadcast_to((128, C, O)),
                )
                if dbg == f"v{t}":
                    nc.sync.dma_start(
                        dbg_d[:, bc * CO : (bc + 1) * CO],
                        v_sb[:, bc * CO : (bc + 1) * CO],
                    )
                if pace and bc == 0:
                    _pace_pe(dummy_ps, xt, PACE_GAP)
            # ---- G[r, i, c, o] = sum_b x[b,r,i] * v[b,c,o] ----
            # per-slot start/stop pairs must stay consecutive: a PSUM
            # region supports only one pending accumulation group
            for i in range(I):
                out_ap = g_ps[i // 3][:, (i % 3) * CO : (i % 3 + 1) * CO]
                for bc in range(2):
                    nc.tensor.matmul(
                        out_ap,
                        xs[:, bc * 1024 + i * 128 : bc * 1024 + (i + 1) * 128],
                        v_sb[:, bc * CO : (bc + 1) * CO],
                        start=(bc == 0),
                        stop=(bc == 1),
                    )
            if pace:
                _pace_pe(dummy_ps, xt, pace_mid)

            # ---- agree[r,c] = sum_{o,i} W[r,c,o,i] * G[r,i,c,o] ----
            # Multiply wst (fp16, (c,i,o) layout) against G read straight
            # from PSUM (fp32) per i-group, as each group's matmuls land.
            tmpA = pool.tile([128, COI], F16, name=f"{rp}tmpA_{t}", tag="tmpA")
            tAv = tmpA[:].rearrange("p (c i o) -> p i c o", c=C, i=I, o=O)
            wstv = wst[:].rearrange("p (c i o) -> p i c o", c=C, i=I, o=O)
            for gg in range(3):
                ni = 3 if gg < 2 else 2
                nc.vector.tensor_mul(
                    tAv[:, gg * 3 : gg * 3 + ni],
                    wstv[:, gg * 3 : gg * 3 + ni],
                    g_ps[gg][:, 0 : ni * CO].rearrange(
                        "p (i c o) -> p i c o", i=ni, c=C, o=O
                    ),
                )
            # fold i-halves (2x packed), then group-reduce per capsule
            tf = pool.tile([128, COI // 2], F16, name=f"{rp}tf_{t}", tag="tf")
            tA4 = tmpA[:].rearrange("p (c i o) -> p c i o", c=C, i=I, o=O)
            nc.vector.tensor_tensor(
                tf[:].rearrange("p (c i o) -> p c i o", c=C, i=I // 2, o=O),
                tA4[:, :, 0 : I // 2], tA4[:, :, I // 2 : I],
                op=mybir.AluOpType.add,
            )
            agree = pool.tile([128, C], F32, name=f"{rp}agree_{t}", tag="agree")
            nc.vector.tensor_reduce(
                agree[:], tf[:].rearrange("p (c io) -> p c io", c=C),
                axis=mybir.AxisListType.X, op=mybir.AluOpType.add,
            )
            if dbg == f"agree{t}":
                nc.sync.dma_start(dbg_d[:, 0 : 2 * C], agree[:].bitcast(F16))
            # ---- b update (raw sums; 1/B folded into the exp scale) ----
            if t == 0:
                # exp reads agree directly; the b_cum copy is issued after
                # the chain tail so it drifts into the AR window
                logits = agree
            else:
                nc.vector.tensor_tensor(
                    b_cum[:], b_cum[:], agree[:], op=mybir.AluOpType.add
                )
                logits = b_cum
            # ---- c = softmax(b/B) over C; wp = c * W ----
            e_sb = pool.tile([RS, C], F32, name=f"{rp}e_{t}", tag="e_sb")
            se = pool.tile([RS, 1], F32, name=f"{rp}se_{t}", tag="se")
            cE = pool.tile([RS, C], F32, name=f"{rp}cE_{t}", tag="cE")
            nc.scalar.activation(
                e_sb[:], logits[:], mybir.ActivationFunctionType.Exp,
                bias=0.0, scale=1.0 / B, accum_out=se[:],
            )
            rse = pool.tile([RS, 1], F32, name=f"{rp}rse_{t}", tag="rse")
            nc.vector.reciprocal(rse[:], se[:])
            nc.vector.tensor_scalar_mul(cE[:], e_sb[:], rse[:])
            # wp[p, i, c, o] = W * c, chunked over i so the first matmul
            # of the next round starts after the first chunk lands
            for half in range(2):
                nc.vector.tensor_mul(
                    wp4[:, half * 4 : (half + 1) * 4],
                    w4[:, half * 4 : (half + 1) * 4],
                    cE[:]
                    .unsqueeze(1)
                    .unsqueeze(3)
                    .broadcast_to((RS, 4, C, O)),
                )
            if t == 0:
                # preserve iteration-0 logits for the t=1 update; runs
                # during the next AR window
                nc.vector.tensor_copy(b_cum[:], agree[:])
        else:
            # ================= final: AllToAll + local reduce ==========
            # A2A shards the [128, 320] payload into 8x16 partition rows;
            # this core ends up with rows 16k..16k+15 of every rank = its
            # 32 batches as [16, (bc, c, o)].  kernel() un-permutes on the
            # host.
            nc.sync.dma_start(a2a_in[:, 0:CO], cat[:, 0:CO])
            nc.scalar.dma_start(a2a_in[:, CO : 2 * CO], cat[:, CO : 2 * CO])
            if fake_cc:
                nc.sync.dma_start(a2a_out[:], a2a_in[:])
            else:
                nc.gpsimd.collective_compute(
                    "AllToAll",
                    mybir.AluOpType.bypass,
                    replica_groups=rg,
                    ins=[a2a_in[:].opt()],
                    outs=[a2a_out[:].opt()],
                )
            if pace and pace_a2a:
                _pace_pe(dummy_ps, xt, pace_a2a)
            FB = 16                      # partition rows per rank shard
            W2 = 2 * CO                  # 320
            # gather the [128, 320] A2A result contiguously (2 rings),
            # then sum the 8 rank shards in ONE matmul against the
            # p%16 one-hot mask: f_ps[j, f] = sum_p mask[p,j]*s_all[p,f]
            s_all16 = pool.tile([128, W2], F16, tag="s_all16")
            for eng, h in ((nc.sync, 0), (nc.scalar, 1)):
                eng.dma_start(
                    s_all16[:, h * CO : (h + 1) * CO],
                    a2a_out[:, h * CO : (h + 1) * CO],
                )
            f_ps = ps.tile([FB, W2], F32, tag="f_ps", name=f"{rp}f_ps")
            nc.tensor.matmul(
                f_ps[:], mask[:], s_all16[:], start=True, stop=True,
            )
            s_f = pool.tile([FB, W2], F16, tag="s_f")
            nc.vector.tensor_copy(s_f[:], f_ps[:])
            # exact squash; sqrt(q) = exp(0.5*ln q) keeps the single table
            g2 = 2 * C
            tq = pool.tile([FB, W2], F32, tag="ftq")
            qf = pool.tile([FB, g2], F32, tag="fq")
            uf = pool.tile([FB, g2], F32, tag="fu")
            dn = pool.tile([FB, g2], F32, tag="fd")
            gff = pool.tile([FB, g2], F32, tag="fg")
            vf = pool.tile([FB, W2], F32, tag="fv")
            nc.vector.tensor_mul(tq[:], s_f[:], s_f[:])
            nc.vector.tensor_reduce(
                qf[:], tq[:].rearrange("p (g o) -> p g o", o=O),
                axis=mybir.AxisListType.X, op=mybir.AluOpType.add,
            )
            nc.scalar.activation(uf[:], qf[:], mybir.ActivationFunctionType.Ln)
            nc.scalar.activation(uf[:], uf[:],
                                 mybir.ActivationFunctionType.Exp, scale=0.5)
            nc.vector.tensor_scalar(
                dn[:], qf[:], 1.0, 1.0, mybir.AluOpType.mult,
                mybir.AluOpType.add,
            )
            rwf = pool.tile([FB, g2], F32, tag="fr")
            nc.vector.reciprocal(rwf[:], dn[:])
            nc.vector.tensor_mul(gff[:], uf[:], rwf[:])
            nc.vector.tensor_mul(
                vf[:].rearrange("p (g o) -> p g o", o=O),
                s_f[:].rearrange("p (g o) -> p g o", o=O),
                gff[:].unsqueeze(2).broadcast_to((FB, g2, O)),
            )
            nc.sync.dma_start(out_d[:], vf[:])


_NC_CACHE = {}


def _get_nc():
    if "nc" not in _NC_CACHE:
        _NC_CACHE["nc"] = build_nc()
    return _NC_CACHE["nc"]


def _get_runner():
    """Compile once; reuse the jitted SPMD callable across kernel() calls."""
    if "runner" in _NC_CACHE:
        return _NC_CACHE["runner"]
    import jax
    from jax.sharding import Mesh, PartitionSpec
    from jax.experimental.shard_map import shard_map
    from concourse import bass2jax

    nc = _get_nc()
    bass2jax.install_neuronx_cc_hook()
    partition_name = (
        nc.partition_id_tensor.name if nc.partition_id_tensor else None
    )
    in_names, out_names, out_avals, zero_outs = [], [], [], []
    for alloc in nc.m.functions[0].allocations:
        if not isinstance(alloc, mybir.MemoryLocationSet):
            continue
        name = alloc.memorylocations[0].name
        if alloc.kind == "ExternalInput":
            if name != partition_name:
                in_names.append(name)
        elif alloc.kind == "ExternalOutput":
            out_names.append(name)
            shape = tuple(alloc.tensor_shape)
            dtype = mybir.dt.np(alloc.dtype)
            out_avals.append(jax.core.ShapedArray(shape, dtype))
            zero_outs.append(np.zeros(shape, dtype))
    n_params = len(in_names)
    n_outs = len(out_avals)
    all_in_names = list(in_names) + list(out_names)
    if partition_name is not None:
        all_in_names.append(partition_name)

    def _body(*args):
        operands = list(args)
        if partition_name is not None:
            operands.append(bass2jax.partition_id_tensor())
        outs = bass2jax._bass_exec_p.bind(
            *operands,
            out_avals=tuple(out_avals),
            in_names=tuple(all_in_names),
            out_names=tuple(out_names),
            lowering_input_output_aliases=(),
            sim_require_finite=True,
            sim_require_nnan=True,
            nc=nc,
        )
        return tuple(outs)

    devices = jax.devices()[:NCORES]
    mesh = Mesh(np.asarray(devices), ("core",))
    in_specs = (PartitionSpec("core"),) * (n_params + n_outs)
    out_specs = (PartitionSpec("core"),) * len(out_names)
    donate = tuple(range(n_params, n_params + n_outs))
    sharded = jax.jit(
        shard_map(_body, mesh=mesh, in_specs=in_specs, out_specs=out_specs,
                  check_rep=False),
        donate_argnums=donate,
        keep_unused=True,
    )

    def run(in_maps):
        concat_in = [
            np.concatenate(
                [np.asarray(in_maps[c][in_names[i]]) for c in range(NCORES)],
                axis=0,
            )
            for i in range(n_params)
        ]
        concat_zeros = [
            np.zeros((NCORES * z.shape[0], *z.shape[1:]), z.dtype)
            for z in zero_outs
        ]
        out_arrs = sharded(*concat_in, *concat_zeros)
        return [
            {
                name: np.asarray(out_arrs[i]).reshape(
                    NCORES, *out_avals[i].shape
                )[c]
                for i, name in enumerate(out_names)
            }
            for c in range(NCORES)
        ]

    _NC_CACHE["runner"] = run
    return run


def make_in_maps(x, W):
    x = np.asarray(x, dtype=np.float32).astype(np.float16)
    W = np.asarray(W, dtype=np.float32).astype(np.float16)
    in_maps = []
    for k in range(NCORES):
        sl = slice(k * RS, (k + 1) * RS)
        x4 = x[:, :, sl].reshape(2, 128, I, RS)          # [bc, b, i, r]
        xs = np.ascontiguousarray(x4.transpose(1, 0, 2, 3)).reshape(
            128, 2 * I * RS
        )                                                # [b, (bc i r)]
        xt = np.ascontiguousarray(x4.transpose(3, 0, 2, 1)).reshape(
            RS, 2 * I * 128
        )                                                # [r, (bc i b)]
        ws = np.ascontiguousarray(W[sl].transpose(0, 3, 1, 2)).reshape(
            RS, COI
        )                                                # [r, (i c o)]
        wst = np.ascontiguousarray(W[sl].transpose(0, 1, 3, 2)).reshape(
            RS, COI
        )                                                # [r, (c i o)]
        msk = np.tile(np.eye(16, dtype=np.float16), (8, 1))
        in_maps.append({"xt": xt, "xs": xs, "ws": ws, "wst": wst, "msk": msk})
    return in_maps


def kernel(x, W):
    in_maps = make_in_maps(x, W)
    results = None
    for attempt in range(2):
        try:
            run = _get_runner()
            results = run(in_maps)
            break
        except Exception:
            # Transient device wedges (NRT_EXEC_UNIT_UNRECOVERABLE) have
            # been observed to recover on a fresh attempt; rebuild the
            # compiled runner once before giving up.
            if attempt == 1:
                raise
            _NC_CACHE.clear()
    # un-permute the A2A shard layout: core k row r half h = batch
    # h*128 + 16k + r
    v = np.stack([r["vout"] for r in results]).reshape(NCORES, 16, 2, CO)
    full = np.empty((B, CO), np.float32)
    for h in range(2):
        full[h * 128 : (h + 1) * 128] = v[:, :, h].reshape(128, CO)
    return full.reshape(B, C, O, 1)


if __name__ == "__main__":
    nc = build_nc()
    print("built ok; instructions:",
          sum(len(bb.instructions) for bb in nc.main_func.blocks))


# revision 23
# speedup vs baseline: 1.0255x; 1.0255x over previous
"""DigitCaps dynamic-routing kernel for 8 Trainium2 NeuronCores.

Strategy (v6): shard the routes dimension R=1024 across the 8 cores (128
routes per core).  u_hat is never materialized: each routing iteration
computes its weighted route-sum

    s[b,c,o] = sum_{r,i} x[b,r,i] * (c_ij[r,c] * W[r,c,o,i])

directly on the PE as 16 accumulating fp16 matmuls (W held i-major so the
moving operand is contiguous).  The per-core partial s is combined across
cores with a single fp16 AllReduce per routing round (iters 0,1) — the
CCE adds on the wire, so the readback is one 80KB download (split across
the sync and scalar HWDGE engines per batch half, letting the bc0 squash
start while bc1 is still landing) and no on-core fold tree is needed.
The final iteration uses an AllToAll; the 8 rank shards it delivers are
summed in ONE PE matmul against a p%16 one-hot mask, and each core
squashes only its own 32-batch output shard.  The agreement update

    agree[r,c] = sum_{b,o,i} W[r,c,o,i] * x[b,r,i] * v[b,c,o]

is computed locally via G[r,i,c,o] = sum_b x[b,r,i]*v[b,c,o] (PE matmuls;
per-PSUM-region start/stop pairs kept consecutive) followed by a multiply
that reads G straight out of PSUM (fp32) against the pre-transposed W and
a group-reduce on the vector engine.

Activation-table discipline: iters 0/1 compute sqrt(q) as exp(0.5*ln q)
so the scalar engine only ever needs the ln/exp table (shared with the
softmax exp).  x is supplied twice from the host (b-major and r-major) so
no on-device transposes are needed.
"""

import math
import sys

for _p in ("/opt/trn_rl_repo",):
    if _p not in sys.path:
        sys.path.insert(0, _p)

import numpy as np

import concourse.bass as bass
import concourse.bacc as bacc
import concourse.mybir as mybir
import concourse.tile as tile
from concourse.bass_utils import run_bass_kernel_spmd

F32 = mybir.dt.float32
F16 = mybir.dt.float16

B, R, C, O, I = 256, 1024, 10, 16, 8
NCORES = 8
RS = R // NCORES          # routes per core
CO = C * O                # 160
COI = C * O * I           # 1280
BS = B // NCORES          # output batch shard per core
NITER = 3
A0 = 0.1                  # softmax(0) over C=10 entries
# pacer counts: keep the PE HAM window busy through collective/DVE
# stretches.  Target (window - ~3.4us HAM grace); overshoot only delays
# the next real matmul by one dummy's tail (~110ns).
PACE_AR = 55              # mm-end .. G-bc0-start window (~10us modeled)
PACE_GAP = 18             # G-bc0 .. G-bc1 gap (~4.5us modeled)
PACE_MID = 30             # agree/softmax stretch (~4us modeled)
PACE_A2A = 0              # nothing runs on PE after the final mm block


def build_nc(reps=1, niter=NITER, fake_cc=False, chain=False, pace=True,
             pace_ar=PACE_AR, pace_mid=PACE_MID, pace_a2a=PACE_A2A, dbg=None):
    nc = bacc.Bacc(
        "TRN2", target_bir_lowering=False, debug=False, num_devices=NCORES
    )
    xt_d = nc.dram_tensor("xt", [RS, 2 * I * 128], F16, kind="ExternalInput")
    xs_d = nc.dram_tensor("xs", [128, 2 * I * RS], F16, kind="ExternalInput")
    # W in (i, c, o) layout: contiguous 160-wide moving operand per i
    ws_d = nc.dram_tensor("ws", [RS, COI], F16, kind="ExternalInput")
    # W in (c, i, o) layout: packed innermost match for the agree multiply
    wst_d = nc.dram_tensor("wst", [RS, COI], F16, kind="ExternalInput")
    # p%16 one-hot fold mask for the final-iteration rank-shard sum
    msk_d = nc.dram_tensor("msk", [128, 16], F16, kind="ExternalInput")
    # [16, 2*CO]: row r = batches (16k + r, 128 + 16k + r) for core k —
    # the A2A partition-shard layout; kernel() un-permutes on the host.
    out_d = nc.dram_tensor("vout", [16, 2 * CO], F32, kind="ExternalOutput")
    dbg_d = (nc.dram_tensor("dbg", [128, 2 * CO], F16, kind="ExternalOutput")
             if dbg else None)

    with tile.TileContext(nc) as tc:
        with (
            tc.tile_pool(name="main", bufs=1) as pool,
            tc.tile_pool(name="ps", bufs=1, space=bass.MemorySpace.PSUM) as ps,
            tc.tile_pool(name="pg", bufs=1, space=bass.MemorySpace.PSUM) as pg,
            tc.tile_pool(name="dram", bufs=1, space="DRAM") as dram,
        ):
            pools = (pool, ps, pg, dram)
            for rep in range(reps):
                _build_body(nc, tc, pools, xt_d, xs_d, ws_d, wst_d, msk_d, out_d, rep,
                            niter=niter, fake_cc=fake_cc,
                            chain=chain and rep > 0, pace=pace,
                            pace_ar=pace_ar, pace_mid=pace_mid,
                            pace_a2a=pace_a2a, dbg=dbg, dbg_d=dbg_d)
    nc.finalize()
    _unify_act_tables(nc)
    return nc


def _unify_act_tables(nc):
    """Point every activation-table load at the one table covering all the
    functions this kernel uses (ln, exp, copy), then drop redundant loads.

    The builtin insertion pass picks the first table containing each
    function (ln -> natural_log, exp -> exp_and_others), which forces a
    ~1.3us table reload at every ln<->exp transition on the scalar engine.
    All loads it inserts are dependency-free queue-order instructions, so
    rewriting ids and deleting duplicates is safe.
    """
    from concourse.hw_specs import get_activation_tables

    need = {
        mybir.ActivationFunctionType.Ln,
        mybir.ActivationFunctionType.Exp,
        mybir.ActivationFunctionType.Copy,
    }
    try:
        tabs = get_activation_tables(nc.m.arch)
        names = list(tabs)
        target = names.index("natural_log_exp_and_others")
        if not need <= tabs["natural_log_exp_and_others"]:
            return
    except Exception:
        # unknown act_info layout: leave the stock (slower) table loads
        return
    cur = None
    for bb in nc.main_func.blocks:
        keep = []
        for inst in bb.instructions:
            if isinstance(inst, mybir.InstLoadActFuncSet):
                inst.act_func_set_id = target
                if cur == target:
                    continue
                cur = target
            elif isinstance(inst, mybir.InstActivation):
                assert inst.func in need, f"unexpected ACT func {inst.func}"
            keep.append(inst)
        bb.instructions[:] = keep


def _build_body(nc, tc, pools, xt_d, xs_d, ws_d, wst_d, msk_d, out_d, rep, niter=NITER,
                fake_cc=False, chain=False, pace=True, pace_ar=PACE_AR,
                pace_mid=PACE_MID, pace_a2a=PACE_A2A, dbg=None, dbg_d=None):
    pool, ps, pg, dram = pools
    rg = [list(range(NCORES))]
    rp = f"r{rep}_"

    def _pace_pe(dummy_ps, xt, n):
        # Keep the PE p-state streak alive across collective/DVE windows:
        # self-paced throwaway matmuls into a scratch PSUM bank.  No
        # consumers; they only read xt, so they fill PE idle time without
        # delaying ready work by more than one dummy's tail (~110ns).
        for _ in range(n):
            nc.tensor.matmul(
                dummy_ps[:], xt[:, 0:128], xt[:, 0:256],
                start=True, stop=True, skip_group_check=True,
            )

    # ---------------- tiles ----------------
    xt = pool.tile([RS, 16 * 128], F16)       # [r, (bc i)*128 + b]
    xs = pool.tile([128, 2 * I * RS], F16)    # [b%128, bc*1024 + i*128 + r]
    ws = pool.tile([RS, COI], F16)            # [r, i*160 + c*16 + o]
    wst = pool.tile([RS, COI], F16)           # [r, c*128 + i*16 + o]
    wp = pool.tile([RS, COI], F16, name=f"{rp}wp", tag="wp")
    prm = pool.tile([1, 2], F32, name=f"{rp}prm", tag="prm")

    if chain:
        # Serialize this rep behind the previous one's final output: a tiny
        # DMA from out_d into xt creates a WAW overlap with the real xt
        # load, so timing reps measure end-to-end latency.
        poison = out_d[0:16, 0:20].bitcast(F16)
        nc.sync.dma_start(xt[0:16, 0 : poison.shape[1]], poison)

    # activation-table prime: first ACT instruction loads the ln/exp table
    nc.vector.memset(prm[:], 1.0)
    nc.scalar.activation(prm[0:1, 0:1], prm[0:1, 0:1],
                         mybir.ActivationFunctionType.Ln)

    # PE warm-up: ~8 throwaway matmuls on a zeroed tile lift the PE out of
    # its cold p-state while the first x/W DMAs land, so the t=0 matmul
    # block starts at speed (overshoot is bounded by one dummy's tail)
    wz = None
    if pace:
        wz = pool.tile([128, 256], F16, name=f"{rp}wz", tag="wz")
        nc.vector.memset(wz[:], 0.0)
        wz_ps = ps.tile([128, 256], F32, tag="dummy_ps", name=f"{rp}wz_ps")
        for _ in range(8):
            nc.tensor.matmul(
                wz_ps[:], wz[:, 0:128], wz[:, 0:256],
                start=True, stop=True, skip_group_check=True,
            )

    nc.sync.dma_start(ws[:], ws_d[:])
    for g in range(4):
        nc.sync.dma_start(
            xt[:, g * 512 : (g + 1) * 512], xt_d[:, g * 512 : (g + 1) * 512]
        )
    # prefetch the G/agree operands now: they stream during the t=0 mm
    # block (which only needs xt/ws) and are long done before AR#1 starts,
    # so the AllReduce window sees no competing SDMA/HBM traffic
    nc.sync.dma_start(xs[:], xs_d[:])
    nc.sync.dma_start(wst[:], wst_d[:])

    # fold mask for the final-iteration rank-shard sum on the PE:
    # mask[p, j] = 1 if p % 16 == j (supplied as a tiny host input)
    mask = pool.tile([128, 16], F16, name=f"{rp}mask", tag="mask")
    nc.sync.dma_start(mask[:], msk_d[:])

    w4 = ws[:].rearrange("p (i c o) -> p i c o", i=I, c=C, o=O)
    wp4 = wp[:].rearrange("p (i c o) -> p i c o", i=I, c=C, o=O)

    dummy_ps = (
        ps.tile([128, 256], F32, tag="dummy_ps", name=f"{rp}dummy_ps")
        if pace else None
    )

    # collective buffers (HBM).  The per-round reduce is ONE fp16
    # AllReduce of the [128, 2*CO] payload: the CCE sums the 8 rank
    # partials on the wire, so the readback is 80KB (vs 640KB for an
    # AllGather + on-core folds) and no fold tree is needed.
    ar_in = [dram.tile([128, 2 * CO], F16, name=f"{rp}ar_in{t}") for t in range(2)]
    ar_out = [
        dram.tile([128, 2 * CO], F16, name=f"{rp}ar_out{t}",
                  addr_space="Shared")
        for t in range(2)
    ]
    a2a_in = dram.tile([128, 2 * CO], F16, name=f"{rp}a2a_in")
    a2a_out = dram.tile([128, 2 * CO], F16, name=f"{rp}a2a_out")

    b_cum = pool.tile([RS, C], F32, name=f"{rp}bcum", tag="bcum")

    if niter == 0:
        nc.sync.dma_start(out_d[:], xt[0:16, 0 : 4 * CO].bitcast(F32))
        return

    for t in range(niter):
        last = t == niter - 1
        # ---- route-weighted sum matmuls ----
        rhs4 = w4 if t == 0 else wp4
        s_ps = [
            ps.tile([128, CO], F32, tag=f"s_ps{bc}", name=f"{rp}s_ps{bc}_{t}")
            for bc in range(2)
        ]
        for bc in range(2):
            for i in range(I):
                nc.tensor.matmul(
                    s_ps[bc][:],
                    xt[:, (bc * 8 + i) * 128 : (bc * 8 + i + 1) * 128],
                    rhs4[:, i],
                    start=(i == 0),
                    stop=(i == I - 1),
                )
        # stage PSUM -> fp16 SBUF on both vector and scalar so the two
        # halves cast in parallel
        cat = pool.tile([128, 2 * CO], F16, tag="cat", name=f"{rp}cat_{t}")
        nc.vector.tensor_copy(cat[:, 0:CO], s_ps[0][:])
        nc.scalar.copy(cat[:, CO : 2 * CO], s_ps[1][:])

        if not last:
            # ================= AllReduce (CCE adds on the wire) ========
            # upload bc0 on sync, bc1 on scalar: parallel descriptor
            # issue (one DMA_DIRECT2D costs ~0.6us of sequencer time)
            nc.sync.dma_start(ar_in[t][:, 0:CO], cat[:, 0:CO])
            nc.scalar.dma_start(ar_in[t][:, CO : 2 * CO], cat[:, CO : 2 * CO])
            if fake_cc:
                nc.sync.dma_start(ar_out[t][:], ar_in[t][:])
            else:
                nc.gpsimd.collective_compute(
                    "AllReduce",
                    mybir.AluOpType.add,
                    replica_groups=rg,
                    ins=[ar_in[t][:].opt()],
                    outs=[ar_out[t][:].opt()],
                )
            if pace and not fake_cc:
                _pace_pe(dummy_ps, xt, pace_ar)
            # download the summed s in bc halves on both DMA engines so
            # squash-bc0 starts while bc1 is still landing
            s_sb = pool.tile([128, 2 * CO], F16, tag="s_sb", name=f"{rp}ssb_{t}")
            nc.sync.dma_start(s_sb[:, 0:CO], ar_out[t][:, 0:CO])
            nc.scalar.dma_start(s_sb[:, CO : 2 * CO], ar_out[t][:, CO : 2 * CO])

            # per-bc: gather(sync+scalar halves) -> fold tree -> squash
            # -> v half -> G matmuls; bc0's chain hides under AG-bc1
            a2 = A0 * A0 if t == 0 else 1.0
            v_sb = pool.tile([128, 2 * CO], F16, tag="v_sb", name=f"{rp}v_{t}")
            if a2 != 1.0:
                lnb = pool.tile([128, 1], F32, name=f"{rp}lnb_{t}", tag="lnb")
                nc.vector.memset(lnb[:], float(math.log(a2)))
            g_ps = [
                pg.tile([128, 3 * CO], F32, tag=f"g_ps{gg}",
                        name=f"{rp}g_ps{gg}_{t}")
                for gg in range(3)
            ]
            for bc in range(2):
                sbc = s_sb[:, bc * CO : (bc + 1) * CO]
                # squash half: v = s * a2*sqrt(q)/(1 + a2*q), q = sum_o s^2
                tsq = pool.tile([128, CO], F32, tag=f"tsq_{bc}",
                                name=f"{rp}tsq_{bc}_{t}")
                q = pool.tile([128, C], F32, tag=f"sq_q{bc}",
                              name=f"{rp}q_{bc}_{t}")
                u2 = pool.tile([128, C], F32, tag=f"sq_u{bc}",
                               name=f"{rp}u_{bc}_{t}")
                den = pool.tile([128, C], F32, tag=f"sq_d{bc}",
                                name=f"{rp}d_{bc}_{t}")
                gf = pool.tile([128, C], F32, tag=f"sq_g{bc}",
                               name=f"{rp}g_{bc}_{t}")
                if dbg == f"s{t}":
                    nc.sync.dma_start(dbg_d[:, bc * CO : (bc + 1) * CO], sbc)
                nc.vector.tensor_mul(tsq[:], sbc, sbc)
                nc.vector.tensor_reduce(
                    q[:], tsq[:].rearrange("p (g o) -> p g o", o=O),
                    axis=mybir.AxisListType.X, op=mybir.AluOpType.add,
                )
                nc.scalar.activation(u2[:], q[:],
                                     mybir.ActivationFunctionType.Ln)
                nc.scalar.activation(
                    u2[:], u2[:], mybir.ActivationFunctionType.Exp,
                    bias=(0.0 if a2 == 1.0 else lnb[:]), scale=0.5,
                )
                nc.vector.tensor_scalar(
                    den[:], q[:], a2, 1.0, mybir.AluOpType.mult,
                    mybir.AluOpType.add,
                )
                rw = pool.tile([128, C], F32, tag=f"sq_r{bc}",
                               name=f"{rp}r_{bc}_{t}")
                nc.vector.reciprocal(rw[:], den[:])
                nc.vector.tensor_mul(gf[:], u2[:], rw[:])
                nc.vector.tensor_mul(
                    v_sb[:, bc * CO : (bc + 1) * CO].rearrange(
                        "p (g o) -> p g o", o=O
                    ),
                    sbc.rearrange("p (g o) -> p g o", o=O),
                    gf[:].unsqueeze(2).broadcast_to((128, C, O)),
                )
                if dbg == f"v{t}":
                    nc.sync.dma_start(
                        dbg_d[:, bc * CO : (bc + 1) * CO],
                        v_sb[:, bc * CO : (bc + 1) * CO],
                    )
                if pace and bc == 0:
                    _pace_pe(dummy_ps, xt, PACE_GAP)
            # ---- G[r, i, c, o] = sum_b x[b,r,i] * v[b,c,o] ----
            # per-slot start/stop pairs must stay consecutive: a PSUM
            # region supports only one pending accumulation group
            for i in range(I):
                out_ap = g_ps[i // 3][:, (i % 3) * CO : (i % 3 + 1) * CO]
                for bc in range(2):
                    nc.tensor.matmul(
                        out_ap,
                        xs[:, bc * 1024 + i * 128 : bc * 1024 + (i + 1) * 128],
                        v_sb[:, bc * CO : (bc + 1) * CO],
                        start=(bc == 0),
                        stop=(bc == 1),
                    )
            if pace:
                _pace_pe(dummy_ps, xt, pace_mid)

            # ---- agree[r,c] = sum_{o,i} W[r,c,o,i] * G[r,i,c,o] ----
            # Multiply wst (fp16, (c,i,o) layout) against G read straight
            # from PSUM (fp32) per i-group, as each group's matmuls land.
            tmpA = pool.tile([128, COI], F16, name=f"{rp}tmpA_{t}", tag="tmpA")
            tAv = tmpA[:].rearrange("p (c i o) -> p i c o", c=C, i=I, o=O)
            wstv = wst[:].rearrange("p (c i o) -> p i c o", c=C, i=I, o=O)
            for gg in range(3):
                ni = 3 if gg < 2 else 2
                nc.vector.tensor_mul(
                    tAv[:, gg * 3 : gg * 3 + ni],
                    wstv[:, gg * 3 : gg * 3 + ni],
                    g_ps[gg][:, 0 : ni * CO].rearrange(
                        "p (i c o) -> p i c o", i=ni, c=C, o=O
                    ),
                )
            # fold i-halves (2x packed), then group-reduce per capsule
            tf = pool.tile([128, COI // 2], F16, name=f"{rp}tf_{t}", tag="tf")
            tA4 = tmpA[:].rearrange("p (c i o) -> p c i o", c=C, i=I, o=O)
            nc.vector.tensor_tensor(
                tf[:].rearrange("p (c i o) -> p c i o", c=C, i=I // 2, o=O),
                tA4[:, :, 0 : I // 2], tA4[:, :, I // 2 : I],
                op=mybir.AluOpType.add,
            )
            agree = pool.tile([128, C], F32, name=f"{rp}agree_{t}", tag="agree")
            nc.vector.tensor_reduce(
                agree[:], tf[:].rearrange("p (c io) -> p c io", c=C),
                axis=mybir.AxisListType.X, op=mybir.AluOpType.add,
            )
            if dbg == f"agree{t}":
                nc.sync.dma_start(dbg_d[:, 0 : 2 * C], agree[:].bitcast(F16))
            # ---- b update (raw sums; 1/B folded into the exp scale) ----
            if t == 0:
                # exp reads agree directly; the b_cum copy is issued after
                # the chain tail so it drifts into the AR window
                logits = agree
            else:
                nc.vector.tensor_tensor(
                    b_cum[:], b_cum[:], agree[:], op=mybir.AluOpType.add
                )
                logits = b_cum
            # ---- c = softmax(b/B) over C; wp = c * W ----
            e_sb = pool.tile([RS, C], F32, name=f"{rp}e_{t}", tag="e_sb")
            se = pool.tile([RS, 1], F32, name=f"{rp}se_{t}", tag="se")
            cE = pool.tile([RS, C], F32, name=f"{rp}cE_{t}", tag="cE")
            nc.scalar.activation(
                e_sb[:], logits[:], mybir.ActivationFunctionType.Exp,
                bias=0.0, scale=1.0 / B,
            )
            # softmax sum on the vector engine: skips the scalar
            # ACTIVATION_READ_ACCUMULATOR round trip (~0.6us of hops)
            nc.vector.tensor_reduce(
                se[:], e_sb[:], axis=mybir.AxisListType.X,
                op=mybir.AluOpType.add,
            )
            rse = pool.tile([RS, 1], F32, name=f"{rp}rse_{t}", tag="rse")
            nc.vector.reciprocal(rse[:], se[:])
            nc.vector.tensor_scalar_mul(cE[:], e_sb[:], rse[:])
            # wp[p, i, c, o] = W * c, chunked over i so the first matmul
            # of the next round starts after the first chunk lands
            for half in range(2):
                nc.vector.tensor_mul(
                    wp4[:, half * 4 : (half + 1) * 4],
                    w4[:, half * 4 : (half + 1) * 4],
                    cE[:]
                    .unsqueeze(1)
                    .unsqueeze(3)
                    .broadcast_to((RS, 4, C, O)),
                )
            if t == 0:
                # preserve iteration-0 logits for the t=1 update; runs
                # during the next AR window
                nc.vector.tensor_copy(b_cum[:], agree[:])
        else:
            # ================= final: AllToAll + local reduce ==========
            # A2A shards the [128, 320] payload into 8x16 partition rows;
            # this core ends up with rows 16k..16k+15 of every rank = its
            # 32 batches as [16, (bc, c, o)].  kernel() un-permutes on the
            # host.
            nc.sync.dma_start(a2a_in[:, 0:CO], cat[:, 0:CO])
            nc.scalar.dma_start(a2a_in[:, CO : 2 * CO], cat[:, CO : 2 * CO])
            if fake_cc:
                nc.sync.dma_start(a2a_out[:], a2a_in[:])
            else:
                nc.gpsimd.collective_compute(
                    "AllToAll",
                    mybir.AluOpType.bypass,
                    replica_groups=rg,
                    ins=[a2a_in[:].opt()],
                    outs=[a2a_out[:].opt()],
                )
            if pace and pace_a2a:
                _pace_pe(dummy_ps, xt, pace_a2a)
            FB = 16                      # partition rows per rank shard
            W2 = 2 * CO                  # 320
            # gather the [128, 320] A2A result contiguously (2 rings),
            # then sum the 8 rank shards in ONE matmul against the
            # p%16 one-hot mask: f_ps[j, f] = sum_p mask[p,j]*s_all[p,f]
            s_all16 = pool.tile([128, W2], F16, tag="s_all16")
            for eng, h in ((nc.sync, 0), (nc.scalar, 1)):
                eng.dma_start(
                    s_all16[:, h * CO : (h + 1) * CO],
                    a2a_out[:, h * CO : (h + 1) * CO],
                )
            f_ps = ps.tile([FB, W2], F32, tag="f_ps", name=f"{rp}f_ps")
            nc.tensor.matmul(
                f_ps[:], mask[:], s_all16[:], start=True, stop=True,
            )
            s_f = pool.tile([FB, W2], F16, tag="s_f")
            nc.vector.tensor_copy(s_f[:], f_ps[:])
            # exact squash; sqrt(q) = exp(0.5*ln q) keeps the single table
            g2 = 2 * C
            tq = pool.tile([FB, W2], F32, tag="ftq")
            qf = pool.tile([FB, g2], F32, tag="fq")
            uf = pool.tile([FB, g2], F32, tag="fu")
            dn = pool.tile([FB, g2], F32, tag="fd")
            gff = pool.tile([FB, g2], F32, tag="fg")
            vf = pool.tile([FB, W2], F32, tag="fv")
            nc.vector.tensor_mul(tq[:], s_f[:], s_f[:])
            nc.vector.tensor_reduce(
                qf[:], tq[:].rearrange("p (g o) -> p g o", o=O),
                axis=mybir.AxisListType.X, op=mybir.AluOpType.add,
            )
            nc.scalar.activation(uf[:], qf[:], mybir.ActivationFunctionType.Ln)
            nc.scalar.activation(uf[:], uf[:],
                                 mybir.ActivationFunctionType.Exp, scale=0.5)
            nc.vector.tensor_scalar(
                dn[:], qf[:], 1.0, 1.0, mybir.AluOpType.mult,
                mybir.AluOpType.add,
            )
            rwf = pool.tile([FB, g2], F32, tag="fr")
            nc.vector.reciprocal(rwf[:], dn[:])
            nc.vector.tensor_mul(gff[:], uf[:], rwf[:])
            nc.vector.tensor_mul(
                vf[:].rearrange("p (g o) -> p g o", o=O),
                s_f[:].rearrange("p (g o) -> p g o", o=O),
                gff[:].unsqueeze(2).broadcast_to((FB, g2, O)),
            )
            nc.sync.dma_start(out_d[:], vf[:])


_NC_CACHE = {}


def _get_nc():
    if "nc" not in _NC_CACHE:
        _NC_CACHE["nc"] = build_nc()
    return _NC_CACHE["nc"]


def _get_runner():
    """Compile once; reuse the jitted SPMD callable across kernel() calls."""
    if "runner" in _NC_CACHE:
        return _NC_CACHE["runner"]
    import jax
    from jax.sharding import Mesh, PartitionSpec
    from jax.experimental.shard_map import shard_map
    from concourse import bass2jax

    nc = _get_nc()
    bass2jax.install_neuronx_cc_hook()
    partition_name = (
        nc.partition_id_tensor.name if nc.partition_id_tensor else None
    )
    in_names, out_names, out_avals, zero_outs = [], [], [], []
    for alloc in nc.m.functions[0].allocations:
        if not isinstance(alloc, mybir.MemoryLocationSet):
            continue
        name = alloc.memorylocations[0].name
        if alloc.kind == "ExternalInput":
            if name != partition_name:
                in_names.append(name)
        elif alloc.kind == "ExternalOutput":
            out_names.append(name)
            shape = tuple(alloc.tensor_shape)
            dtype = mybir.dt.np(alloc.dtype)
            out_avals.append(jax.core.ShapedArray(shape, dtype))
            zero_outs.append(np.zeros(shape, dtype))
    n_params = len(in_names)
    n_outs = len(out_avals)
    all_in_names = list(in_names) + list(out_names)
    if partition_name is not None:
        all_in_names.append(partition_name)

    def _body(*args):
        operands = list(args)
        if partition_name is not None:
            operands.append(bass2jax.partition_id_tensor())
        outs = bass2jax._bass_exec_p.bind(
            *operands,
            out_avals=tuple(out_avals),
            in_names=tuple(all_in_names),
            out_names=tuple(out_names),
            lowering_input_output_aliases=(),
            sim_require_finite=True,
            sim_require_nnan=True,
            nc=nc,
        )
        return tuple(outs)

    devices = jax.devices()[:NCORES]
    mesh = Mesh(np.asarray(devices), ("core",))
    in_specs = (PartitionSpec("core"),) * (n_params + n_outs)
    out_specs = (PartitionSpec("core"),) * len(out_names)
    donate = tuple(range(n_params, n_params + n_outs))
    sharded = jax.jit(
        shard_map(_body, mesh=mesh, in_specs=in_specs, out_specs=out_specs,
                  check_rep=False),
        donate_argnums=donate,
        keep_unused=True,
    )

    def run(in_maps):
        concat_in = [
            np.concatenate(
                [np.asarray(in_maps[c][in_names[i]]) for c in range(NCORES)],
                axis=0,
            )
            for i in range(n_params)
        ]
        concat_zeros = [
            np.zeros((NCORES * z.shape[0], *z.shape[1:]), z.dtype)
            for z in zero_outs
        ]
        out_arrs = sharded(*concat_in, *concat_zeros)
        return [
            {
                name: np.asarray(out_arrs[i]).reshape(
                    NCORES, *out_avals[i].shape
                )[c]
                for i, name in enumerate(out_names)
            }
            for c in range(NCORES)
        ]

    _NC_CACHE["runner"] = run
    return run


def make_in_maps(x, W):
    x = np.asarray(x, dtype=np.float32).astype(np.float16)
    W = np.asarray(W, dtype=np.float32).astype(np.float16)
    in_maps = []
    for k in range(NCORES):
        sl = slice(k * RS, (k + 1) * RS)
        x4 = x[:, :, sl].reshape(2, 128, I, RS)          # [bc, b, i, r]
        xs = np.ascontiguousarray(x4.transpose(1, 0, 2, 3)).reshape(
            128, 2 * I * RS
        )                                                # [b, (bc i r)]
        xt = np.ascontiguousarray(x4.transpose(3, 0, 2, 1)).reshape(
            RS, 2 * I * 128
        )                                                # [r, (bc i b)]
        ws = np.ascontiguousarray(W[sl].transpose(0, 3, 1, 2)).reshape(
            RS, COI
        )                                                # [r, (i c o)]
        wst = np.ascontiguousarray(W[sl].transpose(0, 1, 3, 2)).reshape(
            RS, COI
        )                                                # [r, (c i o)]
        msk = np.tile(np.eye(16, dtype=np.float16), (8, 1))
        in_maps.append({"xt": xt, "xs": xs, "ws": ws, "wst": wst, "msk": msk})
    return in_maps


def kernel(x, W):
    in_maps = make_in_maps(x, W)
    results = None
    for attempt in range(2):
        try:
            run = _get_runner()
            results = run(in_maps)
            break
        except Exception:
            # Transient device wedges (NRT_EXEC_UNIT_UNRECOVERABLE) have
            # been observed to recover on a fresh attempt; rebuild the
            # compiled runner once before giving up.
            if attempt == 1:
                raise
            _NC_CACHE.clear()
    # un-permute the A2A shard layout: core k row r half h = batch
    # h*128 + 16k + r
    v = np.stack([r["vout"] for r in results]).reshape(NCORES, 16, 2, CO)
    full = np.empty((B, CO), np.float32)
    for h in range(2):
        full[h * 128 : (h + 1) * 128] = v[:, :, h].reshape(128, CO)
    return full.reshape(B, C, O, 1)


if __name__ == "__main__":
    nc = build_nc()
    print("built ok; instructions:",
          sum(len(bb.instructions) for bb in nc.main_func.blocks))
